# revision 5
# baseline (speedup 1.0000x reference)
"""Trainium2 Bass kernel for the DAF dual-branch autoregressive forecaster.

Math notes (derived from the reference model):
- Each 3-layer MLP has no intermediate nonlinearity -> collapses to one
  fused linear (W = W0@W1@W2, b = (b0@W1+b1)@W2+b2) followed by LeakyReLU.
- The kernel_size=5 "external" attention has Tq=1, so its logits are
  exp(S - S) = 1 -> softmax is uniform -> it is the mean of v over
  columns [5, T-2]. q/k/pattern are irrelevant for that branch.
- Intermediate autoregressive steps only contribute pred[..., -1:], whose
  last column depends only on that windowed mean of v. So the conv encoder
  and the quadratic attention run ONCE per chain, at the final step T=515.

Sharding: pure data parallel. 32 samples x 2 branches = 64 independent
chains; each of the 8 cores gets 4 src + 4 tgt chains.
"""

import numpy as np
import concourse.bass as bass
import concourse.bacc as bacc
import concourse.tile as tile
from concourse import mybir
from concourse.bass_utils import run_bass_kernel_spmd

AFT = mybir.ActivationFunctionType
ALU = mybir.AluOpType
F32 = mybir.dt.float32

B = 32          # batch
D = 129         # channels (128 + 1)
T0 = 512        # input sequence length
TF = 515        # final-step sequence length
TOUT = 516      # output sequence length
NCORES = 8
NCH = 8         # chains per core (4 src + 4 tgt)
PRED_LEN = 4
SQT = float(np.sqrt(float(TF)))

# s-chunk boundaries for the attention (partition-dim tiling of length TF)
SC = [0, 128, 256, 384, 512, 515]
# free-dim (t) chunks, max 512 for fp32 moving operand / one PSUM bank
def tchunks(lo, hi, step=512):
    out = []
    while lo < hi:
        out.append((lo, min(lo + step, hi)))
        lo = min(lo + step, hi)
    return out


def leaky(nc, ap):
    # in-place LeakyReLU(0.01): x = max(0.01*x, x)   (SBUF only)
    nc.vector.scalar_tensor_tensor(ap, ap, 0.01, ap, op0=ALU.mult, op1=ALU.max)


def build_program():
    nc = bacc.Bacc("TRN2", target_bir_lowering=False, debug=False,
                   num_devices=NCORES)

    def din(name, shape):
        return nc.dram_tensor(name, list(shape), F32, kind="ExternalInput").ap()

    XM = din("xm", (NCH, 128, T0))       # data rows 0..127 per chain
    XE = din("xe", (NCH, T0))            # data row 128 per chain
    W1M = din("w1m", (2, 3, 128, 64))    # conv1 lhsT main per branch/delta
    W1E = din("w1e", (2, 3, 64))         # conv1 channel-128 row per delta
    B1 = din("b1", (2, 64, 1))
    W2M = din("w2m", (2, 5, 64, 128))    # conv2 lhsT per delta
    B2 = din("b2", (2, 128, 1))
    WVM = din("wvm", (2, 128, 128))      # fused value-MLP lhsT main
    WVE = din("wve", (2, 1, 128))        # Wv row 128
    BV = din("bv", (2, 128, 1))
    WQ = din("wq", (128, 128))
    BQ = din("bq", (128, 1))
    WK = din("wk", (128, 128))
    BK = din("bk", (128, 1))
    WO = din("wo", (128, 128))
    BO = din("bo", (128, 1))
    WDM = din("wdm", (2, 128, 128))      # fused dec lhsT, outputs 0..127
    WDL = din("wdl", (2, 128, 1))        # fused dec lhsT, output 128
    BDM = din("bdm", (2, 128, 1))
    BDL = din("bdl", (2, 1, 1))
    EYE = din("eye", (128, 128))

    OUT = nc.dram_tensor("out", [NCH, D, TOUT], F32, kind="ExternalOutput").ap()

    with tile.TileContext(nc) as tc:
        with tc.tile_pool(name="wts", bufs=1) as wts, \
             tc.tile_pool(name="state", bufs=1) as state, \
             tc.tile_pool(name="sb", bufs=2) as sb, \
             tc.tile_pool(name="pmm", bufs=2, space="PSUM") as pmm, \
             tc.tile_pool(name="pacc", bufs=1, space="PSUM") as pacc, \
             tc.tile_pool(name="pcs", bufs=1, space="PSUM") as pcs:

            # ---- load weights ----
            def wtile(name, ap_dram, shape):
                t = wts.tile(list(shape), F32, tag=name)
                nc.sync.dma_start(out=t[:], in_=ap_dram)
                return t

            w1m = wtile("w1m", W1M.rearrange("b d k m -> k (b d) m"), (128, 6, 64))
            w1e = wtile("w1e", W1E.rearrange("b d m -> (b d) m").rearrange(
                "(o n) m -> o n m", o=1), (1, 6, 64))
            b1t = wtile("b1t", B1.rearrange("b k m -> k b m"), (64, 2, 1))
            w2m = wtile("w2m", W2M.rearrange("b d k m -> k (b d) m"), (64, 10, 128))
            b2t = wtile("b2t", B2.rearrange("b k m -> k b m"), (128, 2, 1))
            wvm = wtile("wvm", WVM.rearrange("b k m -> k b m"), (128, 2, 128))
            wve = wtile("wve", WVE.rearrange("b k m -> k b m"), (1, 2, 128))
            bvt = wtile("bvt", BV.rearrange("b k m -> k b m"), (128, 2, 1))
            wq = wtile("wq", WQ, (128, 128))
            bq = wtile("bq", BQ, (128, 1))
            wk = wtile("wk", WK, (128, 128))
            bk = wtile("bk", BK, (128, 1))
            wo = wtile("wo", WO, (128, 128))
            bo = wtile("bo", BO, (128, 1))
            wdm = wtile("wdm", WDM.rearrange("b k m -> k b m"), (128, 2, 128))
            wdl = wtile("wdl", WDL.rearrange("b k m -> k b m"), (128, 2, 1))
            bdm = wtile("bdm", BDM.rearrange("b k m -> k b m"), (128, 2, 1))
            bdl = wtile("bdl", BDL.rearrange("b k m -> k b m"), (1, 2, 1))
            eye = wtile("eye", EYE, (128, 128))
            ones_col = wts.tile([128, 1], F32)
            nc.vector.memset(ones_col[:], 1.0)

            # ---- persistent state ----
            dataM = state.tile([128, NCH, TF], F32)
            dataE = state.tile([1, NCH, TF], F32)   # channel 128 per chain
            vAll = state.tile([128, NCH, TF], F32)
            sums = state.tile([128, NCH], F32)
            c3m = state.tile([128, NCH], F32)       # final-step pred col 515
            c3l = state.tile([1, NCH], F32)

            nc.sync.dma_start(out=dataM[:, :, 0:T0],
                              in_=XM.rearrange("j p t -> p j t"))
            nc.sync.dma_start(out=dataE[0:1, :, 0:T0],
                              in_=XE.rearrange("(o j) t -> o j t", o=1))

            # ---- phase A: v over cols 0..511, per chain; window sums ----
            for j in range(NCH):
                br = j // 4
                pv = pmm.tile([128, TF], F32, tag="mm")
                nc.tensor.matmul(pv[:, 0:T0], wvm[:, br, :], dataM[:, j, 0:T0],
                                 start=True, stop=False)
                nc.tensor.matmul(pv[:, 0:T0], wve[:, br, :], dataE[:, j, 0:T0],
                                 start=False, stop=True)
                nc.scalar.activation(vAll[:, j, 0:T0], pv[:, 0:T0], AFT.Identity,
                                     bias=bvt[:, br, :])
                leaky(nc, vAll[:, j, 0:T0])
                nc.vector.tensor_reduce(sums[:, j:j + 1], vAll[:, j, 5:511],
                                        axis=mybir.AxisListType.X, op=ALU.add)

            # ---- autoregressive steps ----
            for i in range(PRED_LEN):
                cnt = 506 + i
                newcol = T0 + i  # 512+i
                m = sb.tile([128, NCH], F32, tag="arm")
                nc.vector.tensor_scalar_mul(m[:], sums[:], 1.0 / cnt)

                ps8 = pmm.tile([128, NCH], F32, tag="mm")
                nc.tensor.matmul(ps8[:], wo[:], m[:], start=True, stop=True)
                sre = sb.tile([128, NCH], F32, tag="arsre")
                nc.scalar.activation(sre[:], ps8[:], AFT.Identity, bias=bo[:])
                leaky(nc, sre[:])

                pc = pmm.tile([128, NCH], F32, tag="mm")
                pcl = pmm.tile([1, NCH], F32, tag="mm")
                for br in range(2):
                    cs_ = slice(4 * br, 4 * br + 4)
                    nc.tensor.matmul(pc[:, cs_], wdm[:, br, :], sre[:, cs_],
                                     start=True, stop=True)
                    nc.tensor.matmul(pcl[:, cs_], wdl[:, br, :], sre[:, cs_],
                                     start=True, stop=True)
                if i == PRED_LEN - 1:
                    cm, cl = c3m, c3l
                else:
                    cm = sb.tile([128, NCH], F32, tag="arcm")
                    cl = sb.tile([1, NCH], F32, tag="arcl")
                for br in range(2):
                    cs_ = slice(4 * br, 4 * br + 4)
                    nc.scalar.activation(cm[:, cs_], pc[:, cs_], AFT.Identity,
                                         bias=bdm[:, br, :])
                    nc.scalar.activation(cl[:, cs_], pcl[:, cs_], AFT.Identity,
                                         bias=bdl[:, br, :])
                leaky(nc, cm[:])
                leaky(nc, cl[:])

                if i < PRED_LEN - 1:
                    nc.vector.tensor_copy(dataM[:, :, newcol], cm[:])
                    nc.vector.tensor_copy(dataE[0:1, :, newcol], cl[:])
                    # v for the new column
                    pvn = pmm.tile([128, NCH], F32, tag="mm")
                    for br in range(2):
                        cs_ = slice(4 * br, 4 * br + 4)
                        nc.tensor.matmul(pvn[:, cs_], wvm[:, br, :], cm[:, cs_],
                                         start=True, stop=False)
                        nc.tensor.matmul(pvn[:, cs_], wve[:, br, :], cl[:, cs_],
                                         start=False, stop=True)
                        nc.scalar.activation(vAll[:, cs_, newcol], pvn[:, cs_],
                                             AFT.Identity, bias=bvt[:, br, :])
                    leaky(nc, vAll[:, :, newcol])
                    # window for next step gains col 511+i
                    nc.vector.tensor_add(sums[:], sums[:], vAll[:, :, 511 + i])

            # ---- phase B: full forward at T=515 per chain ----
            for j in range(NCH):
                br = j // 4

                # conv1 -> h1 (64, TF)
                h1 = sb.tile([64, TF], F32, tag="h1")
                ph = pmm.tile([64, TF], F32, tag="mm")
                # delta=1 covers every column -> start group
                for (n0, n1) in tchunks(0, TF):
                    nc.tensor.matmul(ph[:, n0:n1], w1m[:, 3 * br + 1, :],
                                     dataM[:, j, n0:n1], start=True, stop=False)
                for dlt, (t_lo, t_hi) in ((0, (1, TF)), (2, (0, TF - 1))):
                    for (n0, n1) in tchunks(t_lo, t_hi):
                        nc.tensor.matmul(
                            ph[:, n0:n1], w1m[:, 3 * br + dlt, :],
                            dataM[:, j, n0 + dlt - 1:n1 + dlt - 1],
                            start=False, stop=False)
                # channel-128 row contributions (K=1 per delta)
                for dlt, (t_lo, t_hi) in ((0, (1, TF)), (1, (0, TF)),
                                          (2, (0, TF - 1))):
                    last = dlt == 2
                    for ci, (n0, n1) in enumerate(tchunks(t_lo, t_hi)):
                        nc.tensor.matmul(
                            ph[:, n0:n1], w1e[:, 3 * br + dlt, :],
                            dataE[:, j, n0 + dlt - 1:n1 + dlt - 1],
                            start=False,
                            stop=(last and ci == len(tchunks(t_lo, t_hi)) - 1))
                nc.scalar.activation(h1[:], ph[:], AFT.Identity,
                                     bias=b1t[:, br, :])

                # conv2 -> pattern (128, TF), leaky
                pattern = sb.tile([128, TF], F32, tag="pattern")
                pp = pmm.tile([128, TF], F32, tag="mm")
                for (n0, n1) in tchunks(0, TF):
                    nc.tensor.matmul(pp[:, n0:n1], w2m[:, 5 * br + 2, :],
                                     h1[:, n0:n1], start=True, stop=False)
                for dlt, (t_lo, t_hi) in ((0, (2, TF)), (1, (1, TF)),
                                          (3, (0, TF - 1)), (4, (0, TF - 2))):
                    last = dlt == 4
                    for ci, (n0, n1) in enumerate(tchunks(t_lo, t_hi)):
                        nc.tensor.matmul(
                            pp[:, n0:n1], w2m[:, 5 * br + dlt, :],
                            h1[:, n0 + dlt - 2:n1 + dlt - 2],
                            start=False,
                            stop=(last and ci == len(tchunks(t_lo, t_hi)) - 1))
                nc.scalar.activation(pattern[:], pp[:], AFT.Identity,
                                     bias=b2t[:, br, :])
                leaky(nc, pattern[:])

                # q, k
                qS = sb.tile([128, TF], F32, tag="qS")
                kS = sb.tile([128, TF], F32, tag="kS")
                for wmat, bvec, dst in ((wq, bq, qS), (wk, bk, kS)):
                    pq = pmm.tile([128, TF], F32, tag="mm")
                    for (n0, n1) in tchunks(0, TF):
                        nc.tensor.matmul(pq[:, n0:n1], wmat[:], pattern[:, n0:n1],
                                         start=True, stop=True)
                    nc.scalar.activation(dst[:], pq[:], AFT.Identity, bias=bvec[:])
                    leaky(nc, dst[:])

                # v^T chunks (s-major) via PE transpose
                vTs = sb.tile([128, 5 * 128], F32, tag="vTs")
                for r in range(5):
                    s0, s1 = SC[r], SC[r + 1]
                    sw = s1 - s0
                    pt = pmm.tile([128, 128], F32, tag="mm")
                    nc.tensor.transpose(pt[0:sw, :], vAll[:, j, s0:s1], eye[:])
                    nc.scalar.activation(vTs[0:sw, 128 * r:128 * r + 128],
                                         pt[0:sw, :], AFT.Copy)

                # attention: S^T chunks -> exp -> diag fix -> exp -> colsum + A@V
                pav = pacc.tile([128, TF], F32, tag="av")
                pc_s = pcs.tile([1, TF], F32, tag="cs")
                for r in range(5):
                    s0, s1 = SC[r], SC[r + 1]
                    sw = s1 - s0
                    pS = pmm.tile([128, TF], F32, tag="mm")
                    for (n0, n1) in tchunks(0, TF):
                        nc.tensor.matmul(pS[0:sw, n0:n1], kS[:, s0:s1],
                                         qS[:, n0:n1], start=True, stop=True)
                    e1 = sb.tile([128, TF], F32, tag="e1")
                    nc.scalar.activation(e1[0:sw, :], pS[0:sw, :], AFT.Exp)
                    # diagonal correction on cols [s0, s1)
                    td = sb.tile([128, 128], F32, tag="td")
                    nc.vector.tensor_mul(td[0:sw, 0:sw], pS[0:sw, s0:s1],
                                         eye[0:sw, 0:sw])
                    nc.scalar.activation(td[0:sw, 0:sw], td[0:sw, 0:sw],
                                         AFT.Exp, scale=-1.0 / SQT)
                    nc.vector.tensor_mul(e1[0:sw, s0:s1], e1[0:sw, s0:s1],
                                         td[0:sw, 0:sw])
                    u = sb.tile([128, TF], F32, tag="u")
                    nc.scalar.activation(u[0:sw, :], e1[0:sw, :], AFT.Exp)
                    first, last = r == 0, r == 4
                    for (n0, n1) in tchunks(0, TF):
                        nc.tensor.matmul(pc_s[0:1, n0:n1], ones_col[0:sw, :],
                                         u[0:sw, n0:n1], start=first, stop=last)
                        nc.tensor.matmul(pav[:, n0:n1],
                                         vTs[0:sw, 128 * r:128 * r + 128],
                                         u[0:sw, n0:n1], start=first, stop=last)

                rc = sb.tile([1, TF], F32, tag="rc")
                nc.vector.reciprocal(rc[:], pc_s[:])
                bc = sb.tile([128, TF], F32, tag="bc")
                nc.gpsimd.partition_broadcast(bc[:], rc[:])
                attnS = sb.tile([128, TF], F32, tag="attnS")
                nc.vector.tensor_mul(attnS[:], pav[:], bc[:])

                # rep_in = leaky(Wo @ attn + bo)
                ri = sb.tile([128, TF], F32, tag="ri")
                pri = pmm.tile([128, TF], F32, tag="mm")
                for (n0, n1) in tchunks(0, TF):
                    nc.tensor.matmul(pri[:, n0:n1], wo[:], attnS[:, n0:n1],
                                     start=True, stop=True)
                nc.scalar.activation(ri[:], pri[:], AFT.Identity, bias=bo[:])
                leaky(nc, ri[:])

                # dec -> pred rows 0..127 and row 128; col 515 from AR step 3
                pm = sb.tile([128, TOUT], F32, tag="pm")
                pl = sb.tile([1, TOUT], F32, tag="pl")
                ppm = pmm.tile([128, TF], F32, tag="mm")
                ppl = pmm.tile([1, TF], F32, tag="mm")
                for (n0, n1) in tchunks(0, TF):
                    nc.tensor.matmul(ppm[:, n0:n1], wdm[:, br, :], ri[:, n0:n1],
                                     start=True, stop=True)
                    nc.tensor.matmul(ppl[:, n0:n1], wdl[:, br, :], ri[:, n0:n1],
                                     start=True, stop=True)
                nc.scalar.activation(pm[:, 0:TF], ppm[:], AFT.Identity,
                                     bias=bdm[:, br, :])
                leaky(nc, pm[:, 0:TF])
                nc.scalar.activation(pl[:, 0:TF], ppl[:], AFT.Identity,
                                     bias=bdl[:, br, :])
                leaky(nc, pl[:, 0:TF])
                nc.vector.tensor_copy(pm[:, TF:TOUT], c3m[:, j:j + 1])
                nc.vector.tensor_copy(pl[:, TF:TOUT], c3l[:, j:j + 1])
                nc.sync.dma_start(out=OUT[j, 0:128, :], in_=pm[:])
                nc.sync.dma_start(out=OUT[j, 128:129, :], in_=pl[:])

    nc.compile()
    return nc


_NC_CACHE = None


def _get_program():
    global _NC_CACHE
    if _NC_CACHE is None:
        _NC_CACHE = build_program()
    return _NC_CACHE


def _fuse_mlp(p):
    W = [np.asarray(w, np.float64) for w in p["W"]]
    b = [np.asarray(x, np.float64) for x in p["b"]]
    Wc = W[0] @ W[1] @ W[2]
    bc = (b[0] @ W[1] + b[1]) @ W[2] + b[2]
    return Wc.astype(np.float32), bc.astype(np.float32)


def _prepare_weights(params):
    f32 = np.float32
    w1m = np.zeros((2, 3, 128, 64), f32)
    w1e = np.zeros((2, 3, 64), f32)
    b1a = np.zeros((2, 64, 1), f32)
    w2m = np.zeros((2, 5, 64, 128), f32)
    b2a = np.zeros((2, 128, 1), f32)
    wvm = np.zeros((2, 128, 128), f32)
    wve = np.zeros((2, 1, 128), f32)
    bva = np.zeros((2, 128, 1), f32)
    wdm = np.zeros((2, 128, 128), f32)
    wdl = np.zeros((2, 128, 1), f32)
    bdm = np.zeros((2, 128, 1), f32)
    bdl = np.zeros((2, 1, 1), f32)
    for br, (enc_key, dec_key) in enumerate(
            (("src_enc", "src_dec"), ("tgt_enc", "tgt_dec"))):
        conv = params[enc_key]["conv"]
        W1 = np.asarray(conv["W1"], f32)   # (64, 129, 3)
        b1 = np.asarray(conv["b1"], f32)
        W2 = np.asarray(conv["W2"], f32)   # (128, 64, 5)
        b2 = np.asarray(conv["b2"], f32)
        for dlt in range(3):
            w1m[br, dlt] = W1[:, 0:128, dlt].T
            w1e[br, dlt] = W1[:, 128, dlt]
        b1a[br] = b1[:, None]
        for dlt in range(5):
            w2m[br, dlt] = W2[:, :, dlt].T
        b2a[br] = b2[:, None]
        Wv, bv = _fuse_mlp(params[enc_key]["v"])     # (129,128),(128,)
        wvm[br] = Wv[0:128]
        wve[br, 0] = Wv[128]
        bva[br] = bv[:, None]
        Wd, bd = _fuse_mlp(params[dec_key])           # (128,129),(129,)
        wdm[br] = Wd[:, 0:128]
        wdl[br] = Wd[:, 128:129]
        bdm[br] = bd[0:128, None]
        bdl[br] = bd[128:129, None]
    Wq, bqv = _fuse_mlp(params["attn"]["q"])
    Wk, bkv = _fuse_mlp(params["attn"]["k"])
    Wo, bov = _fuse_mlp(params["attn"]["o"])
    return dict(
        w1m=w1m, w1e=w1e, b1=b1a, w2m=w2m, b2=b2a, wvm=wvm, wve=wve, bv=bva,
        wq=Wq.astype(f32), bq=bqv[:, None].astype(f32),
        wk=Wk.astype(f32), bk=bkv[:, None].astype(f32),
        wo=Wo.astype(f32), bo=bov[:, None].astype(f32),
        wdm=wdm, wdl=wdl, bdm=bdm, bdl=bdl,
        eye=np.eye(128, dtype=f32),
    )


def kernel(src_data, tgt_data, params):
    src = np.asarray(src_data, np.float32)
    tgt = np.asarray(tgt_data, np.float32)
    wdict = _prepare_weights(params)
    nc = _get_program()

    in_maps = []
    for c in range(NCORES):
        sl = slice(4 * c, 4 * c + 4)
        chains = np.concatenate([src[sl], tgt[sl]], axis=0)  # (8, 129, 512)
        im = dict(wdict)
        im["xm"] = np.ascontiguousarray(chains[:, 0:128, :])
        im["xe"] = np.ascontiguousarray(chains[:, 128, :])
        in_maps.append(im)

    res = run_bass_kernel_spmd(nc, in_maps, list(range(NCORES)))

    src_pred = np.empty((B, D, TOUT), np.float32)
    tgt_pred = np.empty((B, D, TOUT), np.float32)
    for c in range(NCORES):
        out = res.results[c]["out"]  # (8, 129, 516)
        src_pred[4 * c:4 * c + 4] = out[0:4]
        tgt_pred[4 * c:4 * c + 4] = out[4:8]
    return (src_pred, tgt_pred)
